# revision 20
# baseline (speedup 1.0000x reference)
"""Trainium2 Bass kernel for CustomConv2d:
x[32,128,112,112] (f32) * weight[256,128,3,3] + bias[256], stride=1, pad=1
-> out[32,256,112,112] (f32).

Strategy: data-parallel over batch (4 images per core on 8 cores). On each
core the conv is computed as 9 shift-accumulated fp32r matmuls per output
tile: contraction dim = C_IN = 128 (exactly the PE array), weights
stationary [c_in=128, c_out_half=128], moving operand = 4 output rows
(448 spatial positions) of the input image resident in SBUF. The image is
stored zero-padded to 114x114 per partition so every tap is a full-size
matmul whose APs satisfy the fp32r ISA restrictions (even innermost
counts, even outer steps). Weights are pre-transposed on the host to
[c_in, kh, kw, c_out] so every weight DMA and matmul slice is contiguous.
"""

import numpy as np

B, C_IN, H, W = 32, 128, 112, 112
C_OUT, KS = 256, 3
N_CORES = 8
B_SH = B // N_CORES  # images per core
R = 4                # output rows per PSUM tile -> N = R*W = 448 <= 512
HP, WP = H + 2, W + 2  # padded image dims (114 x 114)

_cache = {}


def _build(psum_bufs=8, o_bufs=8, x_bufs=2, with_load=True, with_compute=True,
           with_store=True, x_split=28, taps_subset=None, loop_n=1,
           chunk_group=1):
    import contextlib

    import concourse.mybir as mybir
    from concourse import bacc
    from concourse.tile import TileContext

    f32 = mybir.dt.float32
    f32r = mybir.dt.float32r

    nc = bacc.Bacc("TRN2")
    x_d = nc.dram_tensor("x", [B_SH, C_IN, HP, WP], f32r, kind="ExternalInput")
    w_d = nc.dram_tensor("w", [C_IN, KS * KS * C_OUT], f32r, kind="ExternalInput")
    b_d = nc.dram_tensor("bias", [128, C_OUT // 128], f32, kind="ExternalInput")
    out_d = nc.dram_tensor("out", [B_SH, C_OUT, H, W], f32, kind="ExternalOutput")

    n_half = C_OUT // 128  # 2
    taps = [(dh, dw) for dh in (-1, 0, 1) for dw in (-1, 0, 1)]

    with TileContext(nc) as tc:
        with (
            tc.tile_pool(name="wpool", bufs=1) as wpool,
            tc.tile_pool(name="xpool", bufs=x_bufs) as xpool,
            tc.tile_pool(name="opool", bufs=o_bufs) as opool,
            tc.tile_pool(name="psum", bufs=psum_bufs, space="PSUM") as psum_pool,
        ):
            w_sb = wpool.tile([128, KS * KS * C_OUT], f32r)
            nc.sync.dma_start(out=w_sb[:], in_=w_d[:])
            bias_sb = wpool.tile([128, n_half], f32)
            nc.sync.dma_start(out=bias_sb[:], in_=b_d[:])

            loop_cm = (
                tc.For_i(0, loop_n, 1) if loop_n > 1 else contextlib.nullcontext()
            )
            with loop_cm:
              for b in range(B_SH):
                x_sb = xpool.tile([128, HP * WP], f32r)
                x3 = x_sb[:].rearrange("c (h w) -> c h w", w=WP)
                # image arrives pre-padded (114x114 with zero border) from
                # the host, so this is one fully contiguous DMA (optionally
                # split into x_split pieces for finer prefetch overlap).
                if with_load:
                    xd_flat = x_d[b].rearrange("c h w -> c (h w)")
                    step = HP * WP // x_split
                    for s in range(x_split):
                        lo = s * step
                        hi = HP * WP if s == x_split - 1 else (s + 1) * step
                        nc.sync.dma_start(
                            out=x_sb[:, lo:hi], in_=xd_flat[:, lo:hi]
                        )
                use_taps = taps if taps_subset is None else taps[:taps_subset]
                G = chunk_group
                for h0 in range(0, H, R * G):
                    # G row-chunks x n_half psum banks in flight; tap-inner
                    # ordering reuses each stationary weight tile G times.
                    pss = {}
                    for g in range(G):
                        for m in range(n_half):
                            pss[(g, m)] = psum_pool.tile(
                                [128, R * W], f32, name="ps", tag="ps"
                            )
                    if with_compute:
                        for m in range(n_half):
                            for i, (dh, dw) in enumerate(use_taps):
                                t = (dh + 1) * KS + (dw + 1)
                                co = t * C_OUT + m * 128
                                for g in range(G):
                                    hg = h0 + g * R
                                    nc.tensor.matmul(
                                        pss[(g, m)][:].rearrange(
                                            "p (r w) -> p r w", w=W
                                        ),
                                        w_sb[:, co : co + 128],
                                        x3[:, hg + dh + 1 : hg + dh + 1 + R,
                                           dw + 1 : dw + 1 + W],
                                        start=(i == 0),
                                        stop=(i == len(use_taps) - 1),
                                    )
                    if with_store:
                        for g in range(G):
                            for m in range(n_half):
                                hg = h0 + g * R
                                ps = pss[(g, m)]
                                o_sb = opool.tile([128, R * W], f32)
                                if with_compute:
                                    nc.vector.tensor_scalar_add(
                                        o_sb[:], ps[:], bias_sb[:, m : m + 1]
                                    )
                                else:
                                    nc.vector.tensor_scalar_add(
                                        o_sb[:],
                                        x_sb[:, : R * W].bitcast(f32),
                                        bias_sb[:, m : m + 1],
                                    )
                                nc.sync.dma_start(
                                    out=out_d[b, m * 128 : (m + 1) * 128,
                                              hg : hg + R, :],
                                    in_=o_sb[:],
                                )
    nc.finalize()
    return nc


def _get_nc():
    if "nc" not in _cache:
        _cache["nc"] = _build()
    return _cache["nc"]


def kernel(x, weight, bias, stride=1, padding=1, **_ignored):
    from concourse.bass_utils import run_bass_kernel_spmd

    assert int(stride) == 1 and int(padding) == 1
    x = np.asarray(x, dtype=np.float32)
    weight = np.asarray(weight, dtype=np.float32)
    bias = np.asarray(bias, dtype=np.float32)
    assert x.shape == (B, C_IN, H, W) and weight.shape == (C_OUT, C_IN, KS, KS)
    xp = np.pad(x, ((0, 0), (0, 0), (1, 1), (1, 1)))

    # [c_out, c_in, kh, kw] -> [c_in, kh, kw, c_out] so that the lhsT slice
    # for (tap, half) is contiguous along c_out with c_in on partitions.
    w_t = np.ascontiguousarray(np.transpose(weight, (1, 2, 3, 0))).reshape(
        C_IN, KS * KS * C_OUT
    )
    bias2 = np.ascontiguousarray(bias.reshape(C_OUT // 128, 128).T)

    nc = _get_nc()
    in_maps = [
        {
            "x": np.ascontiguousarray(xp[c * B_SH : (c + 1) * B_SH]),
            "w": w_t,
            "bias": bias2,
        }
        for c in range(N_CORES)
    ]
    res = run_bass_kernel_spmd(nc, in_maps, core_ids=list(range(N_CORES)))
    out = np.concatenate([res.results[c]["out"] for c in range(N_CORES)], axis=0)
    return out


# revision 26
# speedup vs baseline: 1.0569x; 1.0569x over previous
"""Trainium2 Bass kernel for CustomConv2d:
x[32,128,112,112] (f32) * weight[256,128,3,3] + bias[256], stride=1, pad=1
-> out[32,256,112,112] (f32).

Strategy: data-parallel over batch (4 images per core on 8 cores). On each
core the conv is computed as 9 shift-accumulated fp32r matmuls per output
tile: contraction dim = C_IN = 128 (exactly the PE array), weights
stationary [c_in=128, c_out_half=128], moving operand = 4 output rows
(448 spatial positions) of the input image resident in SBUF. The image is
stored zero-padded to 114x114 per partition so every tap is a full-size
matmul whose APs satisfy the fp32r ISA restrictions (even innermost
counts, even outer steps). Weights are pre-transposed on the host to
[c_in, kh, kw, c_out] so every weight DMA and matmul slice is contiguous.

fp32r runs the PE at full rate (1 cycle/row for N>=256, vs 4 for fp32)
with ~1.4e-4 relative error vs the fp32 reference. Per-core roofline:
2016 matmuls x 448 cycles at 2.4 GHz = 376 us compute vs ~235 us DMA
(28 MB in + 51 MB out at ~330 GB/s) -> compute-bound. Measured steady
state ~455 us/pass on TRN2 (in-NEFF loop slope), ~83% of fp32r peak.
Tuned knobs: x DMA split into 28 pieces (prefetch grain), all 8 PSUM
banks in flight, 8 output staging buffers.
"""

import numpy as np

B, C_IN, H, W = 32, 128, 112, 112
C_OUT, KS = 256, 3
N_CORES = 8
B_SH = B // N_CORES  # images per core
R = 4                # output rows per PSUM tile -> N = R*W = 448 <= 512
HP, WP = H + 2, W + 2  # padded image dims (114 x 114)

_cache = {}


def _build(psum_bufs=8, o_bufs=8, x_bufs=2, with_load=True, with_compute=True,
           with_store=True, x_split=28, taps_subset=None, loop_n=1,
           chunk_group=1, loop_stagger=False, store_batch=1):
    import contextlib

    import concourse.mybir as mybir
    from concourse import bacc
    from concourse.tile import TileContext

    f32 = mybir.dt.float32
    f32r = mybir.dt.float32r

    nc = bacc.Bacc("TRN2")
    x_d = nc.dram_tensor("x", [B_SH, C_IN, HP, WP], f32r, kind="ExternalInput")
    w_d = nc.dram_tensor("w", [C_IN, KS * KS * C_OUT], f32r, kind="ExternalInput")
    b_d = nc.dram_tensor("bias", [128, C_OUT // 128], f32, kind="ExternalInput")
    out_d = nc.dram_tensor("out", [B_SH, C_OUT, H, W], f32, kind="ExternalOutput")

    n_half = C_OUT // 128  # 2
    taps = [(dh, dw) for dh in (-1, 0, 1) for dw in (-1, 0, 1)]

    with TileContext(nc) as tc:
        with (
            tc.tile_pool(name="wpool", bufs=1) as wpool,
            tc.tile_pool(name="xpool", bufs=x_bufs) as xpool,
            tc.tile_pool(name="opool", bufs=o_bufs) as opool,
            tc.tile_pool(name="psum", bufs=psum_bufs, space="PSUM") as psum_pool,
        ):
            w_sb = wpool.tile([128, KS * KS * C_OUT], f32r)
            nc.sync.dma_start(out=w_sb[:], in_=w_d[:])
            bias_sb = wpool.tile([128, n_half], f32)
            nc.sync.dma_start(out=bias_sb[:], in_=b_d[:])

            loop_cm = (
                tc.For_i(0, loop_n, 1, staggered_reset=loop_stagger)
                if loop_n > 1
                else contextlib.nullcontext()
            )
            with loop_cm:
              for b in range(B_SH):
                x_sb = xpool.tile([128, HP * WP], f32r)
                x3 = x_sb[:].rearrange("c (h w) -> c h w", w=WP)
                # image arrives pre-padded (114x114 with zero border) from
                # the host, so this is one fully contiguous DMA (optionally
                # split into x_split pieces for finer prefetch overlap).
                if with_load:
                    xd_flat = x_d[b].rearrange("c h w -> c (h w)")
                    step = HP * WP // x_split
                    for s in range(x_split):
                        lo = s * step
                        hi = HP * WP if s == x_split - 1 else (s + 1) * step
                        nc.sync.dma_start(
                            out=x_sb[:, lo:hi], in_=xd_flat[:, lo:hi]
                        )
                use_taps = taps if taps_subset is None else taps[:taps_subset]
                G = chunk_group
                SB = store_batch
                assert (H // R) % (SB * G) == 0 or SB == 1
                osb = {}
                for h0 in range(0, H, R * G):
                    # G row-chunks x n_half psum banks in flight; tap-inner
                    # ordering reuses each stationary weight tile G times.
                    pss = {}
                    for g in range(G):
                        for m in range(n_half):
                            pss[(g, m)] = psum_pool.tile(
                                [128, R * W], f32, name="ps", tag="ps"
                            )
                    if with_compute:
                        for m in range(n_half):
                            for i, (dh, dw) in enumerate(use_taps):
                                t = (dh + 1) * KS + (dw + 1)
                                co = t * C_OUT + m * 128
                                for g in range(G):
                                    hg = h0 + g * R
                                    nc.tensor.matmul(
                                        pss[(g, m)][:].rearrange(
                                            "p (r w) -> p r w", w=W
                                        ),
                                        w_sb[:, co : co + 128],
                                        x3[:, hg + dh + 1 : hg + dh + 1 + R,
                                           dw + 1 : dw + 1 + W],
                                        start=(i == 0),
                                        stop=(i == len(use_taps) - 1),
                                    )
                    if with_store:
                        for g in range(G):
                            for m in range(n_half):
                                hg = h0 + g * R
                                ps = pss[(g, m)]
                                j = (hg // R) % SB
                                if j == 0:
                                    osb[m] = opool.tile(
                                        [128, SB * R * W], f32,
                                        name="osb", tag="osb",
                                    )
                                dst = osb[m][:, j * R * W : (j + 1) * R * W]
                                if with_compute:
                                    nc.vector.tensor_scalar_add(
                                        dst, ps[:], bias_sb[:, m : m + 1]
                                    )
                                else:
                                    nc.vector.tensor_scalar_add(
                                        dst,
                                        x_sb[:, : R * W].bitcast(f32),
                                        bias_sb[:, m : m + 1],
                                    )
                                if j == SB - 1:
                                    nc.sync.dma_start(
                                        out=out_d[
                                            b, m * 128 : (m + 1) * 128,
                                            hg - (SB - 1) * R : hg + R, :,
                                        ],
                                        in_=osb[m][:],
                                    )
    nc.finalize()
    return nc


def _get_nc():
    if "nc" not in _cache:
        _cache["nc"] = _build()
    return _cache["nc"]


def kernel(x, weight, bias, stride=1, padding=1, **_ignored):
    from concourse.bass_utils import run_bass_kernel_spmd

    assert int(stride) == 1 and int(padding) == 1
    x = np.asarray(x, dtype=np.float32)
    weight = np.asarray(weight, dtype=np.float32)
    bias = np.asarray(bias, dtype=np.float32)
    assert x.shape == (B, C_IN, H, W) and weight.shape == (C_OUT, C_IN, KS, KS)
    xp = np.pad(x, ((0, 0), (0, 0), (1, 1), (1, 1)))

    # [c_out, c_in, kh, kw] -> [c_in, kh, kw, c_out] so that the lhsT slice
    # for (tap, half) is contiguous along c_out with c_in on partitions.
    w_t = np.ascontiguousarray(np.transpose(weight, (1, 2, 3, 0))).reshape(
        C_IN, KS * KS * C_OUT
    )
    bias2 = np.ascontiguousarray(bias.reshape(C_OUT // 128, 128).T)

    nc = _get_nc()
    in_maps = [
        {
            "x": np.ascontiguousarray(xp[c * B_SH : (c + 1) * B_SH]),
            "w": w_t,
            "bias": bias2,
        }
        for c in range(N_CORES)
    ]
    res = run_bass_kernel_spmd(nc, in_maps, core_ids=list(range(N_CORES)))
    out = np.concatenate([res.results[c]["out"] for c in range(N_CORES)], axis=0)
    return out
